# revision 40
# baseline (speedup 1.0000x reference)
"""GQA FlashAttention (RMSNorm QK + RoPE, causal) on 8 TRN2 NeuronCores.

Sharding: tensor-parallel over heads (core c owns q-heads 4c..4c+3 and
kv-head c; the GQA group is fully local so attention needs no
collective). Attention output is normalized on the producing core
(denominators come free from a ones-column appended to V; the
reciprocal is a fast DVE approximation broadcast across the head dim
by a rank-1 matmul), then re-sharded head-parallel -> row-parallel
with TWO AllToAlls (one per 1024-row half) so the first collective and
the first half of the out-projection overlap with the attention
compute of the second half. Each core then multiplies its 256 output
rows against the full Wo held in SBUF (bf16, prefetched during the
projection phase).

The projection matmuls for block j+1 are emitted interleaved into the
attention tile loop of block j: attention is scalar-engine(exp)-bound,
and the interleave keeps the tensor engine busy enough that the HAM
clock gate stays at full rate. rsqrt is computed as exp(-0.5*ln(x)) so
the scalar engine stays on ONE activation table set for the whole
kernel. All matmuls run in bf16 (fp32 PSUM accumulate); everything is
computed in the transposed layout (head_dim on partitions) so the
scores output IS the P^T operand the PV matmul needs. The RMSNorm
weights and the rotate-half signs are folded into the cos/sin tables
host-side.
"""

import sys

sys.path.insert(0, "/opt/trn_rl_repo")

import ml_dtypes
import numpy as np
import concourse.bass as bass  # noqa: F401
import concourse.tile as tile
from concourse import mybir, bacc
from concourse.bass_utils import run_bass_kernel_spmd

N_CORES = 8
D_IN = 2048
SEQ = 2048
N_HEADS = 32
N_KV = 8
HD = 64
HPC = N_HEADS // N_CORES  # 4 q heads per core
EPS = 1e-6
NEG = -1.0e9

F32 = mybir.dt.float32
BF16 = mybir.dt.bfloat16
BFNP = ml_dtypes.bfloat16

KT = D_IN // 128  # 16 contraction tiles for projections
QB = 512  # q block
NQB = SEQ // QB  # 4
NKT = SEQ // 128  # 16 kv tiles
ROWS = 128  # output rows per core per half
AF = mybir.ActivationFunctionType


class _OneActSetBacc(bacc.Bacc):
    """Bacc whose activation-table pass maps every activation function to
    the natural_log_exp_and_others set (exp/ln/square/copy all live there),
    so the scalar engine loads its table exactly once instead of thrashing
    between the exp and natural-log sets on every rsqrt."""

    def insert_act_table_loads(self):
        import bass_rust
        from concourse import mybir as _mybir
        from concourse.hw_specs import get_activation_tables

        has_activation = any(
            isinstance(i, _mybir.InstActivation)
            for b in self.main_func.blocks
            for i in b.instructions
        )
        if not has_activation:
            return
        tables = [
            (name, fns if name == "natural_log_exp_and_others" else set())
            for name, fns in get_activation_tables(self.m.arch).items()
        ]
        bass_rust.insert_act_table_loads(self, tables)


def _build():
    nc = _OneActSetBacc(num_devices=N_CORES)

    # x re-tiled host-side: xq[p, j, k, c] = x[512j+c, 128k+p]
    xq = nc.dram_tensor("xq", [128, NQB, KT, QB], BF16, kind="ExternalInput")
    wq = nc.dram_tensor("wq", [128, KT, HPC * HD], BF16, kind="ExternalInput")
    wkv = nc.dram_tensor("wkv", [128, KT, 2 * HD], BF16, kind="ExternalInput")
    wo = nc.dram_tensor("wo", [128, KT, D_IN], BF16, kind="ExternalInput")
    cosq = nc.dram_tensor("cosq", [128, SEQ], BF16, kind="ExternalInput")
    sinq = nc.dram_tensor("sinq", [128, SEQ], BF16, kind="ExternalInput")
    cosk = nc.dram_tensor("cosk", [64, SEQ], BF16, kind="ExternalInput")
    sink = nc.dram_tensor("sink", [64, SEQ], BF16, kind="ExternalInput")
    tri = nc.dram_tensor("tri", [128, 128], F32, kind="ExternalInput")
    onesblk_in = nc.dram_tensor("onesblk", [128, 128], BF16, kind="ExternalInput")
    ident_in = nc.dram_tensor("ident", [64, 64], BF16, kind="ExternalInput")

    out = nc.dram_tensor("out", [2 * ROWS, D_IN], F32, kind="ExternalOutput")

    with tile.TileContext(nc) as tc:
        with (
            tc.tile_pool(name="pers", bufs=1) as pers,
            tc.tile_pool(name="dram", bufs=1, space="DRAM") as dram,
            tc.tile_pool(name="xp", bufs=2) as xp,
            tc.tile_pool(name="psproj", bufs=1, space="PSUM") as psProj,
            tc.tile_pool(name="psaux", bufs=1, space="PSUM") as psAux,
            tc.tile_pool(name="pspo", bufs=1, space="PSUM") as psPo,
            tc.tile_pool(name="pssc", bufs=2, space="PSUM") as psSc,
            tc.tile_pool(name="pspv", bufs=1, space="PSUM") as psPv,
            tc.tile_pool(name="t1", bufs=2) as t1,
            tc.tile_pool(name="t3", bufs=3) as t3,
            tc.tile_pool(name="t5", bufs=2) as t5,
        ):
            # ---------------- persistent SBUF ----------------
            # load order = need order: wq + x block 0 gate the first matmul
            wq_sb = pers.tile([128, KT, HPC * HD], BF16)  # 1 MB
            nc.sync.dma_start(wq_sb[:], wq[:])
            xts = [None] * NQB
            xts[0] = xp.tile([128, KT, QB], BF16, tag="xt", name="xt_0")
            nc.sync.dma_start(xts[0][:], xq[:, 0])
            wkv_sb = pers.tile([128, KT, 2 * HD], BF16)  # 0.5 MB
            nc.sync.dma_start(wkv_sb[:], wkv[:])
            cosq_sb = pers.tile([128, SEQ], BF16)
            sinq_sb = pers.tile([128, SEQ], BF16)
            cosk_sb = pers.tile([64, SEQ], BF16)
            sink_sb = pers.tile([64, SEQ], BF16)
            nc.sync.dma_start(cosq_sb[:], cosq[:])
            nc.sync.dma_start(sinq_sb[:], sinq[:])
            nc.sync.dma_start(cosk_sb[:], cosk[:])
            nc.sync.dma_start(sink_sb[:], sink[:])
            tri_sb = pers.tile([128, 128], F32)
            nc.sync.dma_start(tri_sb[:], tri[:])
            onesblk = pers.tile([128, 128], BF16)
            nc.sync.dma_start(onesblk[:], onesblk_in[:])
            ident = pers.tile([64, 64], BF16)
            nc.sync.dma_start(ident[:], ident_in[:])

            eps_sb = pers.tile([128, 1], F32)
            nc.vector.memset(eps_sb[:], EPS)

            qt_sb = pers.tile([64, HPC, SEQ], BF16)  # 1 MB
            kt_sb = pers.tile([64, SEQ], BF16)
            vaug_sb = pers.tile([128, NKT, HD + 1], BF16)
            nc.vector.memset(vaug_sb[:, :, HD : HD + 1], 1.0)

            # wo prefetch (8 MB bf16), in 4 chunks
            wo_sb = pers.tile([128, KT, D_IN], BF16)
            for wch in range(4):
                nc.sync.dma_start(
                    wo_sb[:, 4 * wch : 4 * wch + 4, :], wo[:, 4 * wch : 4 * wch + 4, :]
                )

            # DRAM scratch for the two AllToAlls (rows = head dims, head h
            # at 64h..64h+64; attention output is pre-normalized)
            a2a_in = [
                dram.tile([N_CORES, 4 * HD, ROWS], BF16, name=f"a2a_in{i}")
                for i in range(2)
            ]
            a2a_out = [
                dram.tile([N_CORES, 4 * HD, ROWS], BF16, name=f"a2a_out{i}")
                for i in range(2)
            ]

            # normalized attnT (head-dim-major) per half, filled by DMA
            an_raw = [
                pers.tile([128, 2 * N_CORES, ROWS], BF16, name=f"an_raw{i}")
                for i in range(2)
            ]

            def p5_prep(half):
                """Pull this core's 128 rows (all 2048 head dims) from the
                AllToAll result."""
                for g in range(N_CORES):
                    nc.sync.dma_start(
                        an_raw[half][:, 2 * g : 2 * g + 2, :],
                        a2a_out[half][g].rearrange("(u p) r -> p u r", u=2),
                    )

            def p5_matmul(half, nb):
                """One 512-col block of the out-projection for one half."""
                osl = slice(512 * nb, 512 * nb + 512)
                po = psPo.tile([128, 512], F32, tag="po", name=f"po_{half}_{nb}")
                for gh in range(2 * N_CORES):
                    nc.tensor.matmul(
                        po[:],
                        an_raw[half][:, gh, :],
                        wo_sb[:, gh, osl],
                        start=(gh == 0),
                        stop=(gh == 2 * N_CORES - 1),
                    )
                osb = t5.tile([128, 512], F32, tag="osb", name=f"osb_{half}_{nb}")
                nc.vector.tensor_copy(osb[:], po[:])
                nc.sync.dma_start(out[128 * half : 128 * half + 128, osl], osb[:])

            def emit_proj(j):
                """Projection matmul emitters for block j (3 accs x 16 k)."""
                acc = [
                    psProj.tile([128, QB], F32, tag="acc0", name=f"acc0_{j}"),
                    psProj.tile([128, QB], F32, tag="acc1", name=f"acc1_{j}"),
                    psProj.tile([128, QB], F32, tag="acc2", name=f"acc2_{j}"),
                ]
                xt = xts[j]
                ops = []
                for k in range(KT):
                    st = k == 0
                    sp = k == KT - 1
                    ops.append(
                        lambda k=k, st=st, sp=sp: (
                            nc.tensor.matmul(
                                acc[0][:], wq_sb[:, k, 0:128], xt[:, k, :],
                                start=st, stop=sp,
                            ),
                            nc.tensor.matmul(
                                acc[1][:], wq_sb[:, k, 128:256], xt[:, k, :],
                                start=st, stop=sp,
                            ),
                            nc.tensor.matmul(
                                acc[2][:], wkv_sb[:, k, :], xt[:, k, :],
                                start=st, stop=sp,
                            ),
                        )
                    )
                return acc, ops

            def norm_rope(j, acc):
                """RMSNorm + RoPE for block j (kv first: attention block j
                needs K before the later q heads)."""
                sl = slice(QB * j, QB * j + QB)
                for idx in (2, 0, 1):
                    raw = acc[idx]
                    is_kv = idx == 2
                    nr = 64 if is_kv else 128
                    rows = slice(0, nr)
                    sq = t1.tile([128, QB], BF16, tag="sq", name=f"sq_{j}_{idx}")
                    nc.scalar.activation(sq[rows, :], raw[rows, :], AF.Square)
                    psn = psAux.tile([128, QB], F32, tag="aux", name=f"psn_{j}_{idx}")
                    nc.tensor.matmul(
                        psn[rows, :], onesblk[rows, rows], sq[rows, :],
                        start=True, stop=True,
                    )
                    # rsqrt(ms + eps) = exp(-0.5 * ln(ms + eps)); one ACT table set
                    lnt = t1.tile([128, QB], F32, tag="lnt", name=f"lnt_{j}_{idx}")
                    nc.scalar.activation(
                        lnt[rows, :], psn[rows, :], AF.Ln, bias=eps_sb[rows, :],
                        scale=1.0 / HD,
                    )
                    rcp = t1.tile([128, QB], F32, tag="rcp", name=f"rcp_{j}_{idx}")
                    nc.scalar.activation(rcp[rows, :], lnt[rows, :], AF.Exp, scale=-0.5)
                    tn = t1.tile([128, QB], BF16, tag="tn", name=f"tn_{j}_{idx}")
                    nc.vector.tensor_mul(tn[rows, :], raw[rows, :], rcp[rows, :])
                    # rotate-half (signs folded into the sin tables)
                    rot = t1.tile([128, QB], BF16, tag="rot", name=f"rot_{j}_{idx}")
                    for b in range(1 if is_kv else 2):
                        o = 64 * b
                        nc.vector.tensor_copy(rot[o : o + 32, :], tn[o + 32 : o + 64, :])
                        nc.vector.tensor_copy(rot[o + 32 : o + 64, :], tn[o : o + 32, :])
                    cw = cosk_sb[:, sl] if is_kv else cosq_sb[rows, sl]
                    sw = sink_sb[:, sl] if is_kv else sinq_sb[rows, sl]
                    tmpc = t1.tile([128, QB], BF16, tag="tmpc", name=f"tmpc_{j}_{idx}")
                    nc.vector.tensor_mul(tmpc[rows, :], tn[rows, :], cw)
                    nc.vector.tensor_mul(rot[rows, :], rot[rows, :], sw)
                    if is_kv:
                        nc.vector.tensor_add(kt_sb[:, sl], tmpc[0:64, :], rot[0:64, :])
                        # V: evict + transpose to kv-major layout
                        vt = t1.tile([64, QB], BF16, tag="vt", name=f"vt_{j}")
                        nc.vector.tensor_copy(vt[:], raw[64:128, :])
                        for ttl in range(QB // 128):
                            tg = (QB // 128) * j + ttl
                            psv = psAux.tile(
                                [128, HD], BF16, tag="aux", name=f"psv_{tg}"
                            )
                            nc.tensor.transpose(
                                psv[:], vt[:, 128 * ttl : 128 * ttl + 128], ident[:]
                            )
                            nc.vector.tensor_copy(vaug_sb[:, tg, 0:HD], psv[:])
                    else:
                        for b in range(2):
                            nc.vector.tensor_add(
                                qt_sb[:, 2 * idx + b, sl],
                                tmpc[64 * b : 64 * b + 64, :],
                                rot[64 * b : 64 * b + 64, :],
                            )

            def attention(j, filler):
                """Attention block j; pops PE filler ops (next block's
                projections) between tiles to keep the tensor engine dense."""
                ntile = (QB // 128) * (j + 1)
                half = j // 2
                s0 = (QB // ROWS) * (j % 2)
                n_stops = HPC * ntile
                fi = 0
                stop = 0
                for h in range(HPC):
                    pv = psPv.tile([HD + 1, QB], F32, tag="pv", name=f"pv_{j}_{h}")
                    for t in range(ntile):
                        diag_m = t - (QB // 128) * j
                        ks = slice(128 * t, 128 * t + 128)
                        if diag_m < 0:
                            n0 = 0
                            qs = slice(QB * j, QB * j + QB)
                        else:
                            n0 = 128 * diag_m
                            qs = slice(QB * j + n0, QB * j + QB)
                        W = QB - n0
                        ps_s = psSc.tile([128, QB], F32, tag="sc", name=f"sc_{j}_{h}_{t}")
                        nc.tensor.matmul(
                            ps_s[:, 0:W], kt_sb[:, ks], qt_sb[:, h, qs],
                            start=True, stop=True,
                        )
                        if diag_m >= 0:
                            nc.vector.tensor_add(
                                ps_s[:, 0:128], ps_s[:, 0:128], tri_sb[:]
                            )
                        pt = t3.tile([128, QB], BF16, tag="pt", name=f"pt_{j}_{h}_{t}")
                        nc.scalar.activation(
                            pt[:, 0:W], ps_s[:, 0:W], AF.Exp, scale=0.125
                        )
                        nc.tensor.matmul(
                            pv[0 : HD + 1, n0:QB], vaug_sb[:, t, :], pt[:, 0:W],
                            start=(t == 0), stop=(t == ntile - 1),
                        )
                        stop += 1
                        while fi < len(filler) and fi * n_stops < stop * len(filler):
                            filler[fi]()
                            fi += 1
                    # normalize on the producing core: denominator reciprocal
                    # (fast approx) broadcast across the head dim by a rank-1
                    # matmul, then one multiply
                    dcp = t3.tile([1, QB], F32, tag="dcp", name=f"dcp_{j}_{h}")
                    nc.vector.tensor_copy(dcp[:], pv[HD : HD + 1, :])
                    rden = t3.tile([1, QB], F32, tag="rden", name=f"rden_{j}_{h}")
                    # (approx-recip's const operands live at partition 0 - the
                    # input must too)
                    nc.vector.reciprocal_approx_fast(rden[:], dcp[:])
                    rdb = t3.tile([1, QB], BF16, tag="rdb", name=f"rdb_{j}_{h}")
                    nc.vector.tensor_copy(rdb[:], rden[:])
                    bc = psAux.tile([HD, QB], F32, tag="aux", name=f"bc_{j}_{h}")
                    nc.tensor.matmul(
                        bc[:], onesblk[0:1, 0:HD], rdb[:], start=True, stop=True
                    )
                    att = t3.tile([HD, QB], BF16, tag="att", name=f"att_{j}_{h}")
                    nc.vector.tensor_copy(att[:], pv[0:HD, :])
                    atn = t3.tile([HD, QB], BF16, tag="atn", name=f"atn_{j}_{h}")
                    nc.vector.tensor_mul(atn[:], att[:], bc[:])
                    # one DMA per (j, h): 4 shard chunks at once
                    nc.sync.dma_start(
                        a2a_in[half][s0 : s0 + 4, 64 * h : 64 * h + 64, :].transpose(
                            [1, 0, 2]
                        ),
                        atn[:].rearrange("p (cc r) -> p cc r", cc=4),
                    )
                while fi < len(filler):
                    filler[fi]()
                    fi += 1

            # ---------------- pipeline ----------------
            xts[1] = xp.tile([128, KT, QB], BF16, tag="xt", name="xt_1")
            nc.sync.dma_start(xts[1][:], xq[:, 1])
            for j in range(NQB):
                acc_j, ops = emit_proj(j)
                for op in ops:
                    op()
                if j + 2 < NQB:
                    xts[j + 2] = xp.tile(
                        [128, KT, QB], BF16, tag="xt", name=f"xt_{j + 2}"
                    )
                    nc.sync.dma_start(xts[j + 2][:], xq[:, j + 2])
                norm_rope(j, acc_j)
                attention(j, [])
                if j == 1:
                    nc.gpsimd.collective_compute(
                        "AllToAll",
                        mybir.AluOpType.bypass,
                        replica_groups=[list(range(N_CORES))],
                        ins=[a2a_in[0][:].opt()],
                        outs=[a2a_out[0][:].opt()],
                    )
                if j == 2:
                    p5_prep(0)

            # ---------------- tail ----------------
            nc.gpsimd.collective_compute(
                "AllToAll",
                mybir.AluOpType.bypass,
                replica_groups=[list(range(N_CORES))],
                ins=[a2a_in[1][:].opt()],
                outs=[a2a_out[1][:].opt()],
            )
            # half-0 out-projection fills the PE during the second AllToAll;
            # the wait hint stops the scheduler from hoisting it into block 3
            with tc.tile_wait_until(0.2):
                for nb in range(4):
                    p5_matmul(0, nb)
            p5_prep(1)
            for nb in range(4):
                p5_matmul(1, nb)

    nc.compile()
    return nc


_NC_CACHE = None


def _get_nc():
    global _NC_CACHE
    if _NC_CACHE is None:
        _NC_CACHE = _build()
    return _NC_CACHE


def _make_in_maps(x, cos, sin, wq, wk, wv, wo, q_norm_w, k_norm_w):
    x = np.asarray(x, dtype=np.float32)
    cos = np.asarray(cos, dtype=np.float32)
    sin = np.asarray(sin, dtype=np.float32)
    wq = np.asarray(wq, dtype=np.float32)
    wk = np.asarray(wk, dtype=np.float32)
    wv = np.asarray(wv, dtype=np.float32)
    wo = np.asarray(wo, dtype=np.float32)
    qw = np.asarray(q_norm_w, dtype=np.float32)
    kw = np.asarray(k_norm_w, dtype=np.float32)

    # x re-tiled: xq[p, j, k, c] = x[0][512j+c, 128k+p]
    xh = x[0].astype(BFNP)  # [SEQ, D_IN]
    xq_t = np.ascontiguousarray(
        xh.reshape(NQB, QB, KT, 128).transpose(3, 0, 2, 1)
    )  # [128, NQB, KT, QB]

    # weight-folded rope tables (signs of rotate-half folded into sin)
    cosT = cos.T  # [64, SEQ]
    sinT = sin.T
    sgn = np.concatenate([-np.ones(32, np.float32), np.ones(32, np.float32)])

    def fold(w):
        w_rot = np.concatenate([w[32:], w[:32]])
        c64 = cosT * w[:, None]
        s64 = sinT * (sgn * w_rot)[:, None]
        return c64, s64

    qc64, qs64 = fold(qw)
    kc64, ks64 = fold(kw)
    cosq_h = np.ascontiguousarray(np.vstack([qc64, qc64]).astype(BFNP))
    sinq_h = np.ascontiguousarray(np.vstack([qs64, qs64]).astype(BFNP))
    cosk_h = np.ascontiguousarray(kc64.astype(BFNP))
    sink_h = np.ascontiguousarray(ks64.astype(BFNP))

    ii, jj = np.meshgrid(np.arange(128), np.arange(128), indexing="ij")
    tri_h = np.where(ii <= jj, 0.0, NEG).astype(np.float32)  # keep kv<=q
    onesblk_h = np.zeros((128, 128), np.float32)
    onesblk_h[0:64, 0:64] = 1.0
    onesblk_h[64:128, 64:128] = 1.0
    onesblk_h = onesblk_h.astype(BFNP)
    ident_h = np.eye(64, dtype=np.float32).astype(BFNP)

    woh = np.ascontiguousarray(
        wo.reshape(KT, 128, D_IN).transpose(1, 0, 2).astype(BFNP)
    )

    in_maps = []
    for c in range(N_CORES):
        wq_c = wq[:, 256 * c : 256 * c + 256]
        wq_c = np.ascontiguousarray(
            wq_c.reshape(KT, 128, 256).transpose(1, 0, 2).astype(BFNP)
        )
        wkv_c = np.concatenate(
            [wk[:, 64 * c : 64 * c + 64], wv[:, 64 * c : 64 * c + 64]], axis=1
        )
        wkv_c = np.ascontiguousarray(
            wkv_c.reshape(KT, 128, 128).transpose(1, 0, 2).astype(BFNP)
        )
        in_maps.append(
            {
                "xq": xq_t,
                "wq": wq_c,
                "wkv": wkv_c,
                "wo": woh,
                "cosq": cosq_h,
                "sinq": sinq_h,
                "cosk": cosk_h,
                "sink": sink_h,
                "tri": tri_h,
                "onesblk": onesblk_h,
                "ident": ident_h,
            }
        )
    return in_maps


def kernel(x, cos, sin, wq, wk, wv, wo, q_norm_w, k_norm_w):
    in_maps = _make_in_maps(x, cos, sin, wq, wk, wv, wo, q_norm_w, k_norm_w)
    nc = _get_nc()
    res = run_bass_kernel_spmd(nc, in_maps, core_ids=list(range(N_CORES)))
    full = np.empty((SEQ, D_IN), np.float32)
    for c in range(N_CORES):
        oc = res.results[c]["out"]
        full[128 * c : 128 * c + 128] = oc[0:128]
        full[1024 + 128 * c : 1024 + 128 * c + 128] = oc[128:256]
    return full.reshape(1, SEQ, D_IN).astype(np.float32)


# revision 41
# speedup vs baseline: 1.0073x; 1.0073x over previous
"""GQA FlashAttention (RMSNorm QK + RoPE, causal) on 8 TRN2 NeuronCores.

Sharding: tensor-parallel over heads (core c owns q-heads 4c..4c+3 and
kv-head c; the GQA group is fully local so attention needs no
collective). Attention output is normalized on the producing core
(denominators come free from a ones-column appended to V; the
reciprocal is a fast DVE approximation broadcast across the head dim
by a rank-1 matmul), then re-sharded head-parallel -> row-parallel
with TWO AllToAlls (one per 1024-row half) so the first collective and
the first half of the out-projection overlap with the attention
compute of the second half. Each core then multiplies its 256 output
rows against the full Wo held in SBUF (bf16, prefetched during the
projection phase).

The projection matmuls for block j+1 are emitted interleaved into the
attention tile loop of block j: attention is scalar-engine(exp)-bound,
and the interleave keeps the tensor engine busy enough that the HAM
clock gate stays at full rate. rsqrt is computed as exp(-0.5*ln(x)) so
the scalar engine stays on ONE activation table set for the whole
kernel. All matmuls run in bf16 (fp32 PSUM accumulate); everything is
computed in the transposed layout (head_dim on partitions) so the
scores output IS the P^T operand the PV matmul needs. The RMSNorm
weights and the rotate-half signs are folded into the cos/sin tables
host-side.
"""

import sys

sys.path.insert(0, "/opt/trn_rl_repo")

import ml_dtypes
import numpy as np
import concourse.bass as bass  # noqa: F401
import concourse.tile as tile
from concourse import mybir, bacc
from concourse.bass_utils import run_bass_kernel_spmd

N_CORES = 8
D_IN = 2048
SEQ = 2048
N_HEADS = 32
N_KV = 8
HD = 64
HPC = N_HEADS // N_CORES  # 4 q heads per core
EPS = 1e-6
NEG = -1.0e9

F32 = mybir.dt.float32
BF16 = mybir.dt.bfloat16
BFNP = ml_dtypes.bfloat16

KT = D_IN // 128  # 16 contraction tiles for projections
QB = 512  # q block
NQB = SEQ // QB  # 4
NKT = SEQ // 128  # 16 kv tiles
ROWS = 128  # output rows per core per half
AF = mybir.ActivationFunctionType


class _OneActSetBacc(bacc.Bacc):
    """Bacc whose activation-table pass maps every activation function to
    the natural_log_exp_and_others set (exp/ln/square/copy all live there),
    so the scalar engine loads its table exactly once instead of thrashing
    between the exp and natural-log sets on every rsqrt."""

    def insert_act_table_loads(self):
        import bass_rust
        from concourse import mybir as _mybir
        from concourse.hw_specs import get_activation_tables

        has_activation = any(
            isinstance(i, _mybir.InstActivation)
            for b in self.main_func.blocks
            for i in b.instructions
        )
        if not has_activation:
            return
        tables = [
            (name, fns if name == "natural_log_exp_and_others" else set())
            for name, fns in get_activation_tables(self.m.arch).items()
        ]
        bass_rust.insert_act_table_loads(self, tables)


def _build():
    nc = _OneActSetBacc(num_devices=N_CORES)

    # x re-tiled host-side: xq[p, j, k, c] = x[512j+c, 128k+p]
    xq = nc.dram_tensor("xq", [128, NQB, KT, QB], BF16, kind="ExternalInput")
    wq = nc.dram_tensor("wq", [128, KT, HPC * HD], BF16, kind="ExternalInput")
    wkv = nc.dram_tensor("wkv", [128, KT, 2 * HD], BF16, kind="ExternalInput")
    wo = nc.dram_tensor("wo", [128, KT, D_IN], BF16, kind="ExternalInput")
    cosq = nc.dram_tensor("cosq", [128, SEQ], BF16, kind="ExternalInput")
    sinq = nc.dram_tensor("sinq", [128, SEQ], BF16, kind="ExternalInput")
    cosk = nc.dram_tensor("cosk", [64, SEQ], BF16, kind="ExternalInput")
    sink = nc.dram_tensor("sink", [64, SEQ], BF16, kind="ExternalInput")
    tri = nc.dram_tensor("tri", [128, 128], F32, kind="ExternalInput")
    onesblk_in = nc.dram_tensor("onesblk", [128, 128], BF16, kind="ExternalInput")
    ident_in = nc.dram_tensor("ident", [64, 64], BF16, kind="ExternalInput")

    out = nc.dram_tensor("out", [2 * ROWS, D_IN], F32, kind="ExternalOutput")

    with tile.TileContext(nc) as tc:
        with (
            tc.tile_pool(name="pers", bufs=1) as pers,
            tc.tile_pool(name="dram", bufs=1, space="DRAM") as dram,
            tc.tile_pool(name="xp", bufs=2) as xp,
            tc.tile_pool(name="psproj", bufs=1, space="PSUM") as psProj,
            tc.tile_pool(name="psaux", bufs=1, space="PSUM") as psAux,
            tc.tile_pool(name="pssc", bufs=2, space="PSUM") as psSc,
            tc.tile_pool(name="pspv", bufs=2, space="PSUM") as psPv,
            tc.tile_pool(name="t1", bufs=2) as t1,
            tc.tile_pool(name="t3", bufs=3) as t3,
            tc.tile_pool(name="t5", bufs=2) as t5,
        ):
            # ---------------- persistent SBUF ----------------
            # load order = need order: wq + x block 0 gate the first matmul
            wq_sb = pers.tile([128, KT, HPC * HD], BF16)  # 1 MB
            nc.sync.dma_start(wq_sb[:], wq[:])
            xts = [None] * NQB
            xts[0] = xp.tile([128, KT, QB], BF16, tag="xt", name="xt_0")
            nc.sync.dma_start(xts[0][:], xq[:, 0])
            wkv_sb = pers.tile([128, KT, 2 * HD], BF16)  # 0.5 MB
            nc.sync.dma_start(wkv_sb[:], wkv[:])
            cosq_sb = pers.tile([128, SEQ], BF16)
            sinq_sb = pers.tile([128, SEQ], BF16)
            cosk_sb = pers.tile([64, SEQ], BF16)
            sink_sb = pers.tile([64, SEQ], BF16)
            nc.sync.dma_start(cosq_sb[:], cosq[:])
            nc.sync.dma_start(sinq_sb[:], sinq[:])
            nc.sync.dma_start(cosk_sb[:], cosk[:])
            nc.sync.dma_start(sink_sb[:], sink[:])
            tri_sb = pers.tile([128, 128], F32)
            nc.sync.dma_start(tri_sb[:], tri[:])
            onesblk = pers.tile([128, 128], BF16)
            nc.sync.dma_start(onesblk[:], onesblk_in[:])
            ident = pers.tile([64, 64], BF16)
            nc.sync.dma_start(ident[:], ident_in[:])

            eps_sb = pers.tile([128, 1], F32)
            nc.vector.memset(eps_sb[:], EPS)

            qt_sb = pers.tile([64, HPC, SEQ], BF16)  # 1 MB
            kt_sb = pers.tile([64, SEQ], BF16)
            vaug_sb = pers.tile([128, NKT, HD + 1], BF16)
            nc.vector.memset(vaug_sb[:, :, HD : HD + 1], 1.0)

            # wo prefetch (8 MB bf16), in 4 chunks
            wo_sb = pers.tile([128, KT, D_IN], BF16)
            for wch in range(4):
                nc.sync.dma_start(
                    wo_sb[:, 4 * wch : 4 * wch + 4, :], wo[:, 4 * wch : 4 * wch + 4, :]
                )

            # DRAM scratch for the two AllToAlls (rows = head dims, head h
            # at 64h..64h+64; attention output is pre-normalized)
            a2a_in = [
                dram.tile([N_CORES, 4 * HD, ROWS], BF16, name=f"a2a_in{i}")
                for i in range(2)
            ]
            a2a_out = [
                dram.tile([N_CORES, 4 * HD, ROWS], BF16, name=f"a2a_out{i}")
                for i in range(2)
            ]

            # normalized attnT (head-dim-major) per half, filled by DMA
            an_raw = [
                pers.tile([128, 2 * N_CORES, ROWS], BF16, name=f"an_raw{i}")
                for i in range(2)
            ]

            def p5_prep(half):
                """Pull this core's 128 rows (all 2048 head dims) from the
                AllToAll result."""
                for g in range(N_CORES):
                    nc.sync.dma_start(
                        an_raw[half][:, 2 * g : 2 * g + 2, :],
                        a2a_out[half][g].rearrange("(u p) r -> p u r", u=2),
                    )

            def p5_matmul(half, nb):
                """One 512-col block of the out-projection for one half."""
                osl = slice(512 * nb, 512 * nb + 512)
                po = psAux.tile([128, 512], F32, tag="aux", name=f"po_{half}_{nb}")
                for gh in range(2 * N_CORES):
                    nc.tensor.matmul(
                        po[:],
                        an_raw[half][:, gh, :],
                        wo_sb[:, gh, osl],
                        start=(gh == 0),
                        stop=(gh == 2 * N_CORES - 1),
                    )
                osb = t5.tile([128, 512], F32, tag="osb", name=f"osb_{half}_{nb}")
                nc.vector.tensor_copy(osb[:], po[:])
                nc.sync.dma_start(out[128 * half : 128 * half + 128, osl], osb[:])

            def emit_proj(j):
                """Projection matmul emitters for block j (3 accs x 16 k)."""
                acc = [
                    psProj.tile([128, QB], F32, tag="acc0", name=f"acc0_{j}"),
                    psProj.tile([128, QB], F32, tag="acc1", name=f"acc1_{j}"),
                    psProj.tile([128, QB], F32, tag="acc2", name=f"acc2_{j}"),
                ]
                xt = xts[j]
                ops = []
                for k in range(KT):
                    st = k == 0
                    sp = k == KT - 1
                    ops.append(
                        lambda k=k, st=st, sp=sp: (
                            nc.tensor.matmul(
                                acc[0][:], wq_sb[:, k, 0:128], xt[:, k, :],
                                start=st, stop=sp,
                            ),
                            nc.tensor.matmul(
                                acc[1][:], wq_sb[:, k, 128:256], xt[:, k, :],
                                start=st, stop=sp,
                            ),
                            nc.tensor.matmul(
                                acc[2][:], wkv_sb[:, k, :], xt[:, k, :],
                                start=st, stop=sp,
                            ),
                        )
                    )
                return acc, ops

            def norm_rope(j, acc):
                """RMSNorm + RoPE for block j (kv first: attention block j
                needs K before the later q heads)."""
                sl = slice(QB * j, QB * j + QB)
                for idx in (2, 0, 1):
                    raw = acc[idx]
                    is_kv = idx == 2
                    nr = 64 if is_kv else 128
                    rows = slice(0, nr)
                    sq = t1.tile([128, QB], BF16, tag="sq", name=f"sq_{j}_{idx}")
                    nc.scalar.activation(sq[rows, :], raw[rows, :], AF.Square)
                    psn = psAux.tile([128, QB], F32, tag="aux", name=f"psn_{j}_{idx}")
                    nc.tensor.matmul(
                        psn[rows, :], onesblk[rows, rows], sq[rows, :],
                        start=True, stop=True,
                    )
                    # rsqrt(ms + eps) = exp(-0.5 * ln(ms + eps)); one ACT table set
                    lnt = t1.tile([128, QB], F32, tag="lnt", name=f"lnt_{j}_{idx}")
                    nc.scalar.activation(
                        lnt[rows, :], psn[rows, :], AF.Ln, bias=eps_sb[rows, :],
                        scale=1.0 / HD,
                    )
                    rcp = t1.tile([128, QB], F32, tag="rcp", name=f"rcp_{j}_{idx}")
                    nc.scalar.activation(rcp[rows, :], lnt[rows, :], AF.Exp, scale=-0.5)
                    tn = t1.tile([128, QB], BF16, tag="tn", name=f"tn_{j}_{idx}")
                    nc.vector.tensor_mul(tn[rows, :], raw[rows, :], rcp[rows, :])
                    # rotate-half (signs folded into the sin tables)
                    rot = t1.tile([128, QB], BF16, tag="rot", name=f"rot_{j}_{idx}")
                    for b in range(1 if is_kv else 2):
                        o = 64 * b
                        nc.vector.tensor_copy(rot[o : o + 32, :], tn[o + 32 : o + 64, :])
                        nc.vector.tensor_copy(rot[o + 32 : o + 64, :], tn[o : o + 32, :])
                    cw = cosk_sb[:, sl] if is_kv else cosq_sb[rows, sl]
                    sw = sink_sb[:, sl] if is_kv else sinq_sb[rows, sl]
                    tmpc = t1.tile([128, QB], BF16, tag="tmpc", name=f"tmpc_{j}_{idx}")
                    nc.vector.tensor_mul(tmpc[rows, :], tn[rows, :], cw)
                    nc.vector.tensor_mul(rot[rows, :], rot[rows, :], sw)
                    if is_kv:
                        nc.vector.tensor_add(kt_sb[:, sl], tmpc[0:64, :], rot[0:64, :])
                        # V: evict + transpose to kv-major layout
                        vt = t1.tile([64, QB], BF16, tag="vt", name=f"vt_{j}")
                        nc.vector.tensor_copy(vt[:], raw[64:128, :])
                        for ttl in range(QB // 128):
                            tg = (QB // 128) * j + ttl
                            psv = psAux.tile(
                                [128, HD], BF16, tag="aux", name=f"psv_{tg}"
                            )
                            nc.tensor.transpose(
                                psv[:], vt[:, 128 * ttl : 128 * ttl + 128], ident[:]
                            )
                            nc.vector.tensor_copy(vaug_sb[:, tg, 0:HD], psv[:])
                    else:
                        for b in range(2):
                            nc.vector.tensor_add(
                                qt_sb[:, 2 * idx + b, sl],
                                tmpc[64 * b : 64 * b + 64, :],
                                rot[64 * b : 64 * b + 64, :],
                            )

            def attention(j, filler):
                """Attention block j; pops PE filler ops (next block's
                projections) between tiles to keep the tensor engine dense."""
                ntile = (QB // 128) * (j + 1)
                half = j // 2
                s0 = (QB // ROWS) * (j % 2)
                n_stops = HPC * ntile
                fi = 0
                stop = 0
                for h in range(HPC):
                    pv = psPv.tile([HD + 1, QB], F32, tag="pv", name=f"pv_{j}_{h}")
                    for t in range(ntile):
                        diag_m = t - (QB // 128) * j
                        ks = slice(128 * t, 128 * t + 128)
                        if diag_m < 0:
                            n0 = 0
                            qs = slice(QB * j, QB * j + QB)
                        else:
                            n0 = 128 * diag_m
                            qs = slice(QB * j + n0, QB * j + QB)
                        W = QB - n0
                        ps_s = psSc.tile([128, QB], F32, tag="sc", name=f"sc_{j}_{h}_{t}")
                        nc.tensor.matmul(
                            ps_s[:, 0:W], kt_sb[:, ks], qt_sb[:, h, qs],
                            start=True, stop=True,
                        )
                        if diag_m >= 0:
                            nc.vector.tensor_add(
                                ps_s[:, 0:128], ps_s[:, 0:128], tri_sb[:]
                            )
                        pt = t3.tile([128, QB], BF16, tag="pt", name=f"pt_{j}_{h}_{t}")
                        nc.scalar.activation(
                            pt[:, 0:W], ps_s[:, 0:W], AF.Exp, scale=0.125
                        )
                        nc.tensor.matmul(
                            pv[0 : HD + 1, n0:QB], vaug_sb[:, t, :], pt[:, 0:W],
                            start=(t == 0), stop=(t == ntile - 1),
                        )
                        stop += 1
                        while fi < len(filler) and fi * n_stops < stop * len(filler):
                            filler[fi]()
                            fi += 1
                    # normalize on the producing core: denominator reciprocal
                    # (fast approx) broadcast across the head dim by a rank-1
                    # matmul, then one multiply
                    dcp = t3.tile([1, QB], F32, tag="dcp", name=f"dcp_{j}_{h}")
                    nc.vector.tensor_copy(dcp[:], pv[HD : HD + 1, :])
                    rden = t3.tile([1, QB], F32, tag="rden", name=f"rden_{j}_{h}")
                    # (approx-recip's const operands live at partition 0 - the
                    # input must too)
                    nc.vector.reciprocal_approx_fast(rden[:], dcp[:])
                    rdb = t3.tile([1, QB], BF16, tag="rdb", name=f"rdb_{j}_{h}")
                    nc.vector.tensor_copy(rdb[:], rden[:])
                    bc = psAux.tile([HD, QB], F32, tag="aux", name=f"bc_{j}_{h}")
                    nc.tensor.matmul(
                        bc[:], onesblk[0:1, 0:HD], rdb[:], start=True, stop=True
                    )
                    att = t3.tile([HD, QB], BF16, tag="att", name=f"att_{j}_{h}")
                    nc.vector.tensor_copy(att[:], pv[0:HD, :])
                    atn = t3.tile([HD, QB], BF16, tag="atn", name=f"atn_{j}_{h}")
                    nc.vector.tensor_mul(atn[:], att[:], bc[:])
                    # one DMA per (j, h): 4 shard chunks at once
                    nc.sync.dma_start(
                        a2a_in[half][s0 : s0 + 4, 64 * h : 64 * h + 64, :].transpose(
                            [1, 0, 2]
                        ),
                        atn[:].rearrange("p (cc r) -> p cc r", cc=4),
                    )
                while fi < len(filler):
                    filler[fi]()
                    fi += 1

            # ---------------- pipeline ----------------
            xts[1] = xp.tile([128, KT, QB], BF16, tag="xt", name="xt_1")
            nc.sync.dma_start(xts[1][:], xq[:, 1])
            for j in range(NQB):
                acc_j, ops = emit_proj(j)
                for op in ops:
                    op()
                if j + 2 < NQB:
                    xts[j + 2] = xp.tile(
                        [128, KT, QB], BF16, tag="xt", name=f"xt_{j + 2}"
                    )
                    nc.sync.dma_start(xts[j + 2][:], xq[:, j + 2])
                norm_rope(j, acc_j)
                attention(j, [])
                if j == 1:
                    nc.gpsimd.collective_compute(
                        "AllToAll",
                        mybir.AluOpType.bypass,
                        replica_groups=[list(range(N_CORES))],
                        ins=[a2a_in[0][:].opt()],
                        outs=[a2a_out[0][:].opt()],
                    )
                if j == 2:
                    p5_prep(0)

            # ---------------- tail ----------------
            nc.gpsimd.collective_compute(
                "AllToAll",
                mybir.AluOpType.bypass,
                replica_groups=[list(range(N_CORES))],
                ins=[a2a_in[1][:].opt()],
                outs=[a2a_out[1][:].opt()],
            )
            # half-0 out-projection fills the PE during the second AllToAll;
            # the wait hint stops the scheduler from hoisting it into block 3
            with tc.tile_wait_until(0.2):
                for nb in range(4):
                    p5_matmul(0, nb)
            p5_prep(1)
            for nb in range(4):
                p5_matmul(1, nb)

    nc.compile()
    return nc


_NC_CACHE = None


def _get_nc():
    global _NC_CACHE
    if _NC_CACHE is None:
        _NC_CACHE = _build()
    return _NC_CACHE


def _make_in_maps(x, cos, sin, wq, wk, wv, wo, q_norm_w, k_norm_w):
    x = np.asarray(x, dtype=np.float32)
    cos = np.asarray(cos, dtype=np.float32)
    sin = np.asarray(sin, dtype=np.float32)
    wq = np.asarray(wq, dtype=np.float32)
    wk = np.asarray(wk, dtype=np.float32)
    wv = np.asarray(wv, dtype=np.float32)
    wo = np.asarray(wo, dtype=np.float32)
    qw = np.asarray(q_norm_w, dtype=np.float32)
    kw = np.asarray(k_norm_w, dtype=np.float32)

    # x re-tiled: xq[p, j, k, c] = x[0][512j+c, 128k+p]
    xh = x[0].astype(BFNP)  # [SEQ, D_IN]
    xq_t = np.ascontiguousarray(
        xh.reshape(NQB, QB, KT, 128).transpose(3, 0, 2, 1)
    )  # [128, NQB, KT, QB]

    # weight-folded rope tables (signs of rotate-half folded into sin)
    cosT = cos.T  # [64, SEQ]
    sinT = sin.T
    sgn = np.concatenate([-np.ones(32, np.float32), np.ones(32, np.float32)])

    def fold(w):
        w_rot = np.concatenate([w[32:], w[:32]])
        c64 = cosT * w[:, None]
        s64 = sinT * (sgn * w_rot)[:, None]
        return c64, s64

    qc64, qs64 = fold(qw)
    kc64, ks64 = fold(kw)
    cosq_h = np.ascontiguousarray(np.vstack([qc64, qc64]).astype(BFNP))
    sinq_h = np.ascontiguousarray(np.vstack([qs64, qs64]).astype(BFNP))
    cosk_h = np.ascontiguousarray(kc64.astype(BFNP))
    sink_h = np.ascontiguousarray(ks64.astype(BFNP))

    ii, jj = np.meshgrid(np.arange(128), np.arange(128), indexing="ij")
    tri_h = np.where(ii <= jj, 0.0, NEG).astype(np.float32)  # keep kv<=q
    onesblk_h = np.zeros((128, 128), np.float32)
    onesblk_h[0:64, 0:64] = 1.0
    onesblk_h[64:128, 64:128] = 1.0
    onesblk_h = onesblk_h.astype(BFNP)
    ident_h = np.eye(64, dtype=np.float32).astype(BFNP)

    woh = np.ascontiguousarray(
        wo.reshape(KT, 128, D_IN).transpose(1, 0, 2).astype(BFNP)
    )

    in_maps = []
    for c in range(N_CORES):
        wq_c = wq[:, 256 * c : 256 * c + 256]
        wq_c = np.ascontiguousarray(
            wq_c.reshape(KT, 128, 256).transpose(1, 0, 2).astype(BFNP)
        )
        wkv_c = np.concatenate(
            [wk[:, 64 * c : 64 * c + 64], wv[:, 64 * c : 64 * c + 64]], axis=1
        )
        wkv_c = np.ascontiguousarray(
            wkv_c.reshape(KT, 128, 128).transpose(1, 0, 2).astype(BFNP)
        )
        in_maps.append(
            {
                "xq": xq_t,
                "wq": wq_c,
                "wkv": wkv_c,
                "wo": woh,
                "cosq": cosq_h,
                "sinq": sinq_h,
                "cosk": cosk_h,
                "sink": sink_h,
                "tri": tri_h,
                "onesblk": onesblk_h,
                "ident": ident_h,
            }
        )
    return in_maps


def kernel(x, cos, sin, wq, wk, wv, wo, q_norm_w, k_norm_w):
    in_maps = _make_in_maps(x, cos, sin, wq, wk, wv, wo, q_norm_w, k_norm_w)
    nc = _get_nc()
    res = run_bass_kernel_spmd(nc, in_maps, core_ids=list(range(N_CORES)))
    full = np.empty((SEQ, D_IN), np.float32)
    for c in range(N_CORES):
        oc = res.results[c]["out"]
        full[128 * c : 128 * c + 128] = oc[0:128]
        full[1024 + 128 * c : 1024 + 128 * c + 128] = oc[128:256]
    return full.reshape(1, SEQ, D_IN).astype(np.float32)


# revision 48
# speedup vs baseline: 1.0506x; 1.0430x over previous
"""GQA FlashAttention (RMSNorm QK + RoPE, causal) on 8 TRN2 NeuronCores.

Sharding: tensor-parallel over heads (core c owns q-heads 4c..4c+3 and
kv-head c; the GQA group is fully local so attention needs no
collective). The attention output is re-sharded head-parallel ->
row-parallel with TWO AllToAlls (one per 1024-row half) so the first
collective and the first half of the out-projection overlap with the
attention compute of the second half. Each core then multiplies its
256 output rows (2 x 128) against the full Wo held in SBUF (bf16,
prefetched during the projection phase).

Softmax uses the unnormalized-exp trick: denominators come free from a
ones-column appended to V, and the division is applied after the
AllToAll via a select-matmul broadcast. rsqrt/reciprocal are computed
as exp(-a*ln(x)) so the scalar engine stays on ONE activation table
set (natural_log_exp_and_others) for the whole kernel, and the slow
DVE iterative-divide reciprocal is never used.

All matmuls run in bf16 (fp32 PSUM accumulate). Everything is computed
in the transposed layout (head_dim on partitions) so the scores output
IS the P^T operand the PV matmul needs - zero transposes in the
attention inner loop. The RMSNorm weights and the rotate-half signs
are folded into per-row cos/sin tables host-side.
"""

import sys

sys.path.insert(0, "/opt/trn_rl_repo")

import ml_dtypes
import numpy as np
import concourse.bass as bass  # noqa: F401
import concourse.tile as tile
from concourse import mybir, bacc
from concourse.bass_utils import run_bass_kernel_spmd

N_CORES = 8
D_IN = 2048
SEQ = 2048
N_HEADS = 32
N_KV = 8
HD = 64
HPC = N_HEADS // N_CORES  # 4 q heads per core
EPS = 1e-6
NEG = -1.0e9

F32 = mybir.dt.float32
BF16 = mybir.dt.bfloat16
BFNP = ml_dtypes.bfloat16

KT = D_IN // 128  # 16 contraction tiles for projections
QB = 512  # q block
NQB = SEQ // QB  # 4
NKT = SEQ // 128  # 16 kv tiles
ROWS = 128  # output rows per core per half
AF = mybir.ActivationFunctionType


class _OneActSetBacc(bacc.Bacc):
    """Bacc whose activation-table pass maps every activation function to
    the natural_log_exp_and_others set (exp/ln/square/copy all live there),
    so the scalar engine loads its table exactly once instead of thrashing
    between the exp and natural-log sets on every rsqrt."""

    def insert_act_table_loads(self):
        import bass_rust
        from concourse import mybir as _mybir
        from concourse.hw_specs import get_activation_tables

        has_activation = any(
            isinstance(i, _mybir.InstActivation)
            for b in self.main_func.blocks
            for i in b.instructions
        )
        if not has_activation:
            return
        tables = [
            (name, fns if name == "natural_log_exp_and_others" else set())
            for name, fns in get_activation_tables(self.m.arch).items()
        ]
        bass_rust.insert_act_table_loads(self, tables)


def _build():
    nc = _OneActSetBacc(num_devices=N_CORES)

    # x re-tiled host-side: xq[p, j, k, c] = x[512j+c, 128k+p]
    xq = nc.dram_tensor("xq", [128, NQB, KT, QB], BF16, kind="ExternalInput")
    wq = nc.dram_tensor("wq", [128, KT, HPC * HD], BF16, kind="ExternalInput")
    wkv = nc.dram_tensor("wkv", [128, KT, 2 * HD], BF16, kind="ExternalInput")
    wo = nc.dram_tensor("wo", [128, KT, D_IN], BF16, kind="ExternalInput")
    cosq = nc.dram_tensor("cosq", [128, SEQ], BF16, kind="ExternalInput")
    sinq = nc.dram_tensor("sinq", [128, SEQ], BF16, kind="ExternalInput")
    cosk = nc.dram_tensor("cosk", [64, SEQ], BF16, kind="ExternalInput")
    sink = nc.dram_tensor("sink", [64, SEQ], BF16, kind="ExternalInput")
    tri = nc.dram_tensor("tri", [128, 128], F32, kind="ExternalInput")
    onesblk_in = nc.dram_tensor("onesblk", [128, 128], BF16, kind="ExternalInput")
    onescol_in = nc.dram_tensor("onescol", [128, 1], BF16, kind="ExternalInput")
    ident_in = nc.dram_tensor("ident", [64, 64], BF16, kind="ExternalInput")
    sel = nc.dram_tensor("sel", [4 * N_CORES, 2 * N_CORES, 128], BF16, kind="ExternalInput")

    out = nc.dram_tensor("out", [2 * ROWS, D_IN], F32, kind="ExternalOutput")

    with tile.TileContext(nc) as tc:
        with (
            tc.tile_pool(name="pers", bufs=1) as pers,
            tc.tile_pool(name="dram", bufs=1, space="DRAM") as dram,
            tc.tile_pool(name="xp", bufs=2) as xp,
            tc.tile_pool(name="psproj", bufs=1, space="PSUM") as psProj,
            tc.tile_pool(name="psaux", bufs=1, space="PSUM") as psAux,
            tc.tile_pool(name="pssc", bufs=2, space="PSUM") as psSc,
            tc.tile_pool(name="pspv", bufs=2, space="PSUM") as psPv,
            tc.tile_pool(name="t1", bufs=2) as t1,
            tc.tile_pool(name="t3", bufs=3) as t3,
            tc.tile_pool(name="t5", bufs=2) as t5,
        ):
            # ---------------- persistent SBUF ----------------
            # load order = need order: wq + x block 0 gate the first matmul
            wq_sb = pers.tile([128, KT, HPC * HD], BF16)  # 1 MB
            nc.sync.dma_start(wq_sb[:], wq[:])
            xts = [None] * NQB
            xts[0] = xp.tile([128, KT, QB], BF16, tag="xt", name="xt_0")
            nc.sync.dma_start(xts[0][:], xq[:, 0])
            wkv_sb = pers.tile([128, KT, 2 * HD], BF16)  # 0.5 MB
            nc.sync.dma_start(wkv_sb[:], wkv[:])
            cosq_sb = pers.tile([128, SEQ], BF16)
            sinq_sb = pers.tile([128, SEQ], BF16)
            cosk_sb = pers.tile([64, SEQ], BF16)
            sink_sb = pers.tile([64, SEQ], BF16)
            nc.sync.dma_start(cosq_sb[:], cosq[:])
            nc.sync.dma_start(sinq_sb[:], sinq[:])
            nc.sync.dma_start(cosk_sb[:], cosk[:])
            nc.sync.dma_start(sink_sb[:], sink[:])
            tri_sb = pers.tile([128, 128], F32)
            nc.sync.dma_start(tri_sb[:], tri[:])
            onesblk = pers.tile([128, 128], BF16)
            nc.sync.dma_start(onesblk[:], onesblk_in[:])
            ident = pers.tile([64, 64], BF16)
            nc.sync.dma_start(ident[:], ident_in[:])
            sel_sb = pers.tile([4 * N_CORES, 2 * N_CORES, 128], BF16)
            nc.sync.dma_start(sel_sb[:], sel[:])

            eps_sb = pers.tile([128, 1], F32)
            nc.vector.memset(eps_sb[:], EPS)

            qt_sb = pers.tile([64, HPC, SEQ], BF16)  # 1 MB
            kt_sb = pers.tile([64, SEQ], BF16)
            vaug_sb = pers.tile([128, NKT, HD + 1], BF16)
            nc.vector.memset(vaug_sb[:, :, HD : HD + 1], 1.0)

            # wo prefetch (8 MB bf16), in 4 chunks
            wo_sb = pers.tile([128, KT, D_IN], BF16)
            for wch in range(4):
                nc.sync.dma_start(
                    wo_sb[:, 4 * wch : 4 * wch + 4, :], wo[:, 4 * wch : 4 * wch + 4, :]
                )

            # DRAM scratch for the two AllToAlls
            # rows 0:256 = head dims (head h at 64h..64h+64), rows 256:260 = denoms
            a2a_in = [
                dram.tile([N_CORES, 4 * (HD + 1), ROWS], BF16, name=f"a2a_in{i}")
                for i in range(2)
            ]
            a2a_out = [
                dram.tile([N_CORES, 4 * (HD + 1), ROWS], BF16, name=f"a2a_out{i}")
                for i in range(2)
            ]

            an_sb = [None, None]  # normalized attnT per half

            def p5_prep(half):
                """Denominator reciprocals + normalized attnT for one half."""
                # dsb row 8*hh + g = denom of head hh from source core g
                dsb = t5.tile([4 * N_CORES, ROWS], BF16, tag="dsb", name=f"dsb_{half}")
                for hh in range(HPC):
                    nc.sync.dma_start(
                        dsb[8 * hh : 8 * hh + 8, :],
                        a2a_out[half][:, 4 * HD + hh, :],
                    )
                rcd = t5.tile([4 * N_CORES, ROWS], F32, tag="rcd", name=f"rcd_{half}")
                nc.scalar.activation(rcd[:], dsb[:], AF.Ln)
                drc = t5.tile([4 * N_CORES, ROWS], BF16, tag="drc", name=f"drc_{half}")
                nc.scalar.activation(drc[:], rcd[:], AF.Exp, scale=-1.0)

                an = pers.tile([128, 2 * N_CORES, ROWS], BF16, name=f"an_sb_{half}")
                an_sb[half] = an
                for g in range(N_CORES):
                    araw = t5.tile(
                        [128, 2, ROWS], BF16, tag="araw", name=f"araw_{half}_{g}", bufs=4
                    )
                    nc.sync.dma_start(
                        araw[:],
                        a2a_out[half][g, 0 : 4 * HD, :].rearrange(
                            "(u p) r -> p u r", u=2
                        ),
                    )
                    for u in range(2):
                        gh = 2 * g + u
                        bc = psAux.tile([128, ROWS], F32, tag="aux", name=f"bc_{half}_{gh}")
                        nc.tensor.matmul(
                            bc[:], sel_sb[:, gh, :], drc[:], start=True, stop=True
                        )
                        nc.vector.tensor_mul(an[:, gh, :], araw[:, u, :], bc[:])

            def p5_matmul(half, nb):
                """One 512-col block of the out-projection for one half."""
                osl = slice(512 * nb, 512 * nb + 512)
                po = psAux.tile([128, 512], F32, tag="aux", name=f"po_{half}_{nb}")
                an = an_sb[half]
                for gh in range(2 * N_CORES):
                    nc.tensor.matmul(
                        po[:],
                        an[:, gh, :],
                        wo_sb[:, gh, osl],
                        start=(gh == 0),
                        stop=(gh == 2 * N_CORES - 1),
                    )
                osb = t5.tile([128, 512], F32, tag="osb", name=f"osb_{half}_{nb}")
                nc.vector.tensor_copy(osb[:], po[:])
                nc.sync.dma_start(out[128 * half : 128 * half + 128, osl], osb[:])

            # prefetch distance 1: block j+1 loads while block j computes
            xts[1] = xp.tile([128, KT, QB], BF16, tag="xt", name="xt_1")
            nc.sync.dma_start(xts[1][:], xq[:, 1])

            def emit_proj(j):
                """Projection matmul thunks for block j (3 accs x 16 k)."""
                acc = [
                    psProj.tile([128, QB], F32, tag="acc0", name=f"acc0_{j}"),
                    psProj.tile([128, QB], F32, tag="acc1", name=f"acc1_{j}"),
                    psProj.tile([128, QB], F32, tag="acc2", name=f"acc2_{j}"),
                ]
                xt = xts[j]
                ops = []
                for k in range(KT):
                    st = k == 0
                    sp = k == KT - 1
                    ops.append(
                        lambda k=k, st=st, sp=sp: (
                            nc.tensor.matmul(
                                acc[0][:], wq_sb[:, k, 0:128], xt[:, k, :],
                                start=st, stop=sp,
                            ),
                            nc.tensor.matmul(
                                acc[1][:], wq_sb[:, k, 128:256], xt[:, k, :],
                                start=st, stop=sp,
                            ),
                            nc.tensor.matmul(
                                acc[2][:], wkv_sb[:, k, :], xt[:, k, :],
                                start=st, stop=sp,
                            ),
                        )
                    )
                return acc, ops

            def norm_rope(j, acc):
                sl = slice(QB * j, QB * j + QB)
                # kv first: attention needs K before the later q heads
                for idx in (2, 0, 1):
                    raw = acc[idx]
                    is_kv = idx == 2
                    nr = 64 if is_kv else 128
                    rows = slice(0, nr)
                    sq = t1.tile([128, QB], BF16, tag="sq", name=f"sq_{j}_{idx}")
                    nc.scalar.activation(sq[rows, :], raw[rows, :], AF.Square)
                    psn = psAux.tile([128, QB], F32, tag="aux", name=f"psn_{j}_{idx}")
                    nc.tensor.matmul(
                        psn[rows, :], onesblk[rows, rows], sq[rows, :],
                        start=True, stop=True,
                    )
                    # rsqrt(ms + eps) = exp(-0.5 * ln(ms + eps)); one ACT table set
                    lnt = t1.tile([128, QB], F32, tag="lnt", name=f"lnt_{j}_{idx}")
                    nc.scalar.activation(
                        lnt[rows, :], psn[rows, :], AF.Ln, bias=eps_sb[rows, :],
                        scale=1.0 / HD,
                    )
                    rcp = t1.tile([128, QB], F32, tag="rcp", name=f"rcp_{j}_{idx}")
                    nc.scalar.activation(rcp[rows, :], lnt[rows, :], AF.Exp, scale=-0.5)
                    tn = t1.tile([128, QB], BF16, tag="tn", name=f"tn_{j}_{idx}")
                    nc.vector.tensor_mul(tn[rows, :], raw[rows, :], rcp[rows, :])
                    # rotate-half (signs folded into the sin tables)
                    rot = t1.tile([128, QB], BF16, tag="rot", name=f"rot_{j}_{idx}")
                    for b in range(1 if is_kv else 2):
                        o = 64 * b
                        nc.vector.tensor_copy(rot[o : o + 32, :], tn[o + 32 : o + 64, :])
                        nc.vector.tensor_copy(rot[o + 32 : o + 64, :], tn[o : o + 32, :])
                    cw = cosk_sb[:, sl] if is_kv else cosq_sb[rows, sl]
                    sw = sink_sb[:, sl] if is_kv else sinq_sb[rows, sl]
                    tmpc = t1.tile([128, QB], BF16, tag="tmpc", name=f"tmpc_{j}_{idx}")
                    nc.vector.tensor_mul(tmpc[rows, :], tn[rows, :], cw)
                    nc.vector.tensor_mul(rot[rows, :], rot[rows, :], sw)
                    if is_kv:
                        nc.vector.tensor_add(kt_sb[:, sl], tmpc[0:64, :], rot[0:64, :])
                        # V: evict + transpose to kv-major layout
                        vt = t1.tile([64, QB], BF16, tag="vt", name=f"vt_{j}")
                        nc.vector.tensor_copy(vt[:], raw[64:128, :])
                        for ttl in range(QB // 128):
                            tg = (QB // 128) * j + ttl
                            psv = psAux.tile(
                                [128, HD], BF16, tag="aux", name=f"psv_{tg}"
                            )
                            nc.tensor.transpose(
                                psv[:], vt[:, 128 * ttl : 128 * ttl + 128], ident[:]
                            )
                            nc.vector.tensor_copy(vaug_sb[:, tg, 0:HD], psv[:])
                    else:
                        for b in range(2):
                            nc.vector.tensor_add(
                                qt_sb[:, 2 * idx + b, sl],
                                tmpc[64 * b : 64 * b + 64, :],
                                rot[64 * b : 64 * b + 64, :],
                            )

            def attention(j, filler):
                """Attention block j; after each head, a burst of the next
                block's projection matmuls keeps the PE fed while the scalar
                engine works through this block's exps (all of the filler is
                emitted by the end of head 2, so norm j+1 can start)."""
                ntile = (QB // 128) * (j + 1)
                half = j // 2
                den_j = t3.tile([1, HPC, QB], BF16, tag="den", name=f"den_{j}")
                s0 = (QB // ROWS) * (j % 2)
                fi = 0
                for h in range(HPC):
                    pv = psPv.tile([HD + 1, QB], F32, tag="pv", name=f"pv_{j}_{h}")
                    for t in range(ntile):
                        diag_m = t - (QB // 128) * j
                        ks = slice(128 * t, 128 * t + 128)
                        if diag_m < 0:
                            n0 = 0
                            qs = slice(QB * j, QB * j + QB)
                        else:
                            n0 = 128 * diag_m
                            qs = slice(QB * j + n0, QB * j + QB)
                        W = QB - n0
                        ps_s = psSc.tile([128, QB], F32, tag="sc", name=f"sc_{j}_{h}_{t}")
                        nc.tensor.matmul(
                            ps_s[:, 0:W], kt_sb[:, ks], qt_sb[:, h, qs],
                            start=True, stop=True,
                        )
                        if diag_m >= 0:
                            nc.vector.tensor_add(
                                ps_s[:, 0:128], ps_s[:, 0:128], tri_sb[:]
                            )
                        pt = t3.tile([128, QB], BF16, tag="pt", name=f"pt_{j}_{h}_{t}")
                        nc.scalar.activation(
                            pt[:, 0:W], ps_s[:, 0:W], AF.Exp, scale=0.125
                        )
                        nc.tensor.matmul(
                            pv[0 : HD + 1, n0:QB], vaug_sb[:, t, :], pt[:, 0:W],
                            start=(t == 0), stop=(t == ntile - 1),
                        )
                    att = t3.tile([HD, QB], BF16, tag="att", name=f"att_{j}_{h}")
                    nc.vector.tensor_copy(att[:], pv[0:HD, :])
                    nc.vector.tensor_copy(den_j[:, h, :], pv[HD : HD + 1, :])
                    # one DMA per (j, h): 4 shard chunks at once
                    nc.sync.dma_start(
                        a2a_in[half][s0 : s0 + 4, 64 * h : 64 * h + 64, :].transpose(
                            [1, 0, 2]
                        ),
                        att[:].rearrange("p (cc r) -> p cc r", cc=4),
                    )
                    # normalization prep for half 0 lands mid-block-3: its PE/ACT
                    # bits are tiny and all inputs (a2a0) are long since ready
                    if j == 3 and h == 1:
                        p5_prep(0)
                    # next block's projections, spread over heads 0..2
                    target = min(len(filler), ((h + 1) * len(filler) + 2) // 3)
                    while fi < target:
                        filler[fi]()
                        fi += 1
                # denominators for the block (one DMA per head)
                for h in range(HPC):
                    nc.sync.dma_start(
                        a2a_in[half][
                            s0 : s0 + 4, 4 * HD + h : 4 * HD + h + 1, :
                        ].transpose([1, 0, 2]),
                        den_j[:, h, :].rearrange("p (c r) -> p c r", c=4),
                    )

            # ---------------- pipeline ----------------
            acc_j, ops = emit_proj(0)
            for op in ops:
                op()
            for j in range(NQB):
                norm_rope(j, acc_j)
                if j + 1 < NQB:
                    next_acc, filler = emit_proj(j + 1)
                else:
                    next_acc, filler = None, []
                attention(j, filler)
                acc_j = next_acc
                if j + 2 < NQB:
                    xts[j + 2] = xp.tile(
                        [128, KT, QB], BF16, tag="xt", name=f"xt_{j + 2}"
                    )
                    nc.sync.dma_start(xts[j + 2][:], xq[:, j + 2])
                if j == 1:
                    nc.gpsimd.collective_compute(
                        "AllToAll",
                        mybir.AluOpType.bypass,
                        replica_groups=[list(range(N_CORES))],
                        ins=[a2a_in[0][:].opt()],
                        outs=[a2a_out[0][:].opt()],
                    )
            # ---------------- tail ----------------
            nc.gpsimd.collective_compute(
                "AllToAll",
                mybir.AluOpType.bypass,
                replica_groups=[list(range(N_CORES))],
                ins=[a2a_in[1][:].opt()],
                outs=[a2a_out[1][:].opt()],
            )
            # half-0 out-projection fills the PE during the second AllToAll;
            # the wait hint stops the scheduler from hoisting it into block 3
            # (where it would delay the block-3 scores and starve the exps)
            with tc.tile_wait_until(0.2):
                for nb in range(4):
                    p5_matmul(0, nb)
            p5_prep(1)
            for nb in range(4):
                p5_matmul(1, nb)

    nc.compile()
    return nc


_NC_CACHE = None


def _get_nc():
    global _NC_CACHE
    if _NC_CACHE is None:
        _NC_CACHE = _build()
    return _NC_CACHE


def _make_in_maps(x, cos, sin, wq, wk, wv, wo, q_norm_w, k_norm_w):
    x = np.asarray(x, dtype=np.float32)
    cos = np.asarray(cos, dtype=np.float32)
    sin = np.asarray(sin, dtype=np.float32)
    wq = np.asarray(wq, dtype=np.float32)
    wk = np.asarray(wk, dtype=np.float32)
    wv = np.asarray(wv, dtype=np.float32)
    wo = np.asarray(wo, dtype=np.float32)
    qw = np.asarray(q_norm_w, dtype=np.float32)
    kw = np.asarray(k_norm_w, dtype=np.float32)

    # x re-tiled: xq[p, j, k, c] = x[0][512j+c, 128k+p]
    xh = x[0].astype(BFNP)  # [SEQ, D_IN]
    xq_t = np.ascontiguousarray(
        xh.reshape(NQB, QB, KT, 128).transpose(3, 0, 2, 1)
    )  # [128, NQB, KT, QB]

    # weight-folded rope tables (signs of rotate-half folded into sin)
    cosT = cos.T  # [64, SEQ]
    sinT = sin.T
    sgn = np.concatenate([-np.ones(32, np.float32), np.ones(32, np.float32)])

    def fold(w):
        w_rot = np.concatenate([w[32:], w[:32]])
        c64 = cosT * w[:, None]
        s64 = sinT * (sgn * w_rot)[:, None]
        return c64, s64

    qc64, qs64 = fold(qw)
    kc64, ks64 = fold(kw)
    cosq_h = np.ascontiguousarray(np.vstack([qc64, qc64]).astype(BFNP))
    sinq_h = np.ascontiguousarray(np.vstack([qs64, qs64]).astype(BFNP))
    cosk_h = np.ascontiguousarray(kc64.astype(BFNP))
    sink_h = np.ascontiguousarray(ks64.astype(BFNP))

    ii, jj = np.meshgrid(np.arange(128), np.arange(128), indexing="ij")
    tri_h = np.where(ii <= jj, 0.0, NEG).astype(np.float32)  # keep kv<=q
    onesblk_h = np.zeros((128, 128), np.float32)
    onesblk_h[0:64, 0:64] = 1.0
    onesblk_h[64:128, 64:128] = 1.0
    onesblk_h = onesblk_h.astype(BFNP)
    onescol_h = np.ones((128, 1), np.float32).astype(BFNP)
    ident_h = np.eye(64, dtype=np.float32).astype(BFNP)
    sel_h = np.zeros((4 * N_CORES, 2 * N_CORES, 128), np.float32)
    for g in range(N_CORES):
        for u in range(2):
            for m in range(128):
                # dsb row 8*hh + g with hh = 2u + m//64
                sel_h[8 * (2 * u + m // 64) + g, 2 * g + u, m] = 1.0
    sel_h = sel_h.astype(BFNP)

    woh = np.ascontiguousarray(
        wo.reshape(KT, 128, D_IN).transpose(1, 0, 2).astype(BFNP)
    )

    in_maps = []
    for c in range(N_CORES):
        wq_c = wq[:, 256 * c : 256 * c + 256]
        wq_c = np.ascontiguousarray(
            wq_c.reshape(KT, 128, 256).transpose(1, 0, 2).astype(BFNP)
        )
        wkv_c = np.concatenate(
            [wk[:, 64 * c : 64 * c + 64], wv[:, 64 * c : 64 * c + 64]], axis=1
        )
        wkv_c = np.ascontiguousarray(
            wkv_c.reshape(KT, 128, 128).transpose(1, 0, 2).astype(BFNP)
        )
        in_maps.append(
            {
                "xq": xq_t,
                "wq": wq_c,
                "wkv": wkv_c,
                "wo": woh,
                "cosq": cosq_h,
                "sinq": sinq_h,
                "cosk": cosk_h,
                "sink": sink_h,
                "tri": tri_h,
                "onesblk": onesblk_h,
                "onescol": onescol_h,
                "ident": ident_h,
                "sel": sel_h,
            }
        )
    return in_maps


def kernel(x, cos, sin, wq, wk, wv, wo, q_norm_w, k_norm_w):
    in_maps = _make_in_maps(x, cos, sin, wq, wk, wv, wo, q_norm_w, k_norm_w)
    nc = _get_nc()
    res = run_bass_kernel_spmd(nc, in_maps, core_ids=list(range(N_CORES)))
    full = np.empty((SEQ, D_IN), np.float32)
    for c in range(N_CORES):
        oc = res.results[c]["out"]
        full[128 * c : 128 * c + 128] = oc[0:128]
        full[1024 + 128 * c : 1024 + 128 * c + 128] = oc[128:256]
    return full.reshape(1, SEQ, D_IN).astype(np.float32)
